# revision 38
# baseline (speedup 1.0000x reference)
"""Trainium2 (Bass/Tile) kernel for quantized multi-head attention.

Distributed across 8 NeuronCores: tensor-parallel over heads for the
Q4_0-dequant + QKV projections + RoPE + causal attention, one small
AllToAll per batch (overlapped with later batches), then a
token-parallel output projection.

Scheduling notes:
 - dequant is one broadcast-multiply per half-block (host ships nibbles
   pre-widened to int8), halves alternating DVE / GpSimd, then an xbar
   transpose into [c, oc] layout.
 - batch 0 runs q-passes for ts0-3 first so matmuls start as soon as
   wt_q is ready while wt_k / wt_v still dequantize.
 - all eight wo panels are dequantized into a DRAM staging buffer
   during phase-1 slack; phase 2 only re-loads them (no compute on the
   critical path).
 - attention interleaves head pairs per k-tile with the AV matmuls
   trailing one tile (hides the Exp latency); the first-half q-chunks
   run first because they only need the first half of the per-batch
   Q/K/V tiles.  The causal diagonal mask and the softmax denominator
   are computed with tiny matmuls (mask @ I accumulate, ones^T @ ptree).
 - wo matmuls on staged panels interleave into batch-3's attention so
   the final AllToAll is off the critical path.
"""

import math
from dataclasses import dataclass

import numpy as np

import concourse.bass as bass
import concourse.tile as tile
from concourse import bacc, mybir, bass_isa

BF = mybir.dt.bfloat16
FP16 = mybir.dt.float16
F32 = mybir.dt.float32
I8 = mybir.dt.int8
AOP = mybir.AluOpType
AF = mybir.ActivationFunctionType


@dataclass
class Cfg:
    B: int = 4
    S: int = 1024
    D: int = 4096
    NCORES: int = 8
    SCH: int = 512   # kept for test.py compat (unused)
    QCH: int = 512   # attention q-chunk

    @property
    def T(self):
        return self.B * self.S

    @property
    def H(self):
        return self.D // 128  # total heads (head_dim 128)

    @property
    def H_LOC(self):
        return self.H // self.NCORES

    @property
    def C_SHARD(self):
        return self.H_LOC * 128  # local channels

    @property
    def SPC(self):
        return self.S // self.NCORES  # seq slice per core per batch (128)

    @property
    def TPC(self):
        return self.B * self.SPC  # tokens per core (output slice)

    @property
    def NGP(self):
        return self.D // 128  # contraction k-tiles / group-pairs per row


def build_program(cfg: Cfg):
    """Build the per-core Bass program. Returns compiled nc."""
    c = cfg
    assert c.S % c.QCH == 0 and c.QCH <= 512
    assert c.S % (128 * c.NCORES) == 0

    import concourse.tile_utils as tile_utils
    tile_utils.max_sbuf_usage = 208 * 1024

    nc = bacc.Bacc("TRN2", target_bir_lowering=False, debug=False,
                   num_devices=c.NCORES)

    OSH = c.C_SHARD      # qkv weight shard out-channels per core
    NG = 2 * c.NGP       # scale groups (of 64) per out-channel row
    ngp = c.NGP

    # ---- external I/O ----
    # x pre-tiled on host: [p, b*8+ts, g, t]
    x4_d = nc.dram_tensor("x4", [128, c.T // 128, ngp, 128], BF,
                          kind="ExternalInput")
    q8_q = nc.dram_tensor("q8q", [OSH, c.D], I8, kind="ExternalInput")
    q8_k = nc.dram_tensor("q8k", [OSH, c.D], I8, kind="ExternalInput")
    q8_v = nc.dram_tensor("q8v", [OSH, c.D], I8, kind="ExternalInput")
    q8_o = nc.dram_tensor("q8o", [c.D, c.D], I8, kind="ExternalInput")
    s4_q = nc.dram_tensor("s4q", [128, OSH // 128, NG], BF,
                          kind="ExternalInput")
    s4_k = nc.dram_tensor("s4k", [128, OSH // 128, NG], BF,
                          kind="ExternalInput")
    s4_v = nc.dram_tensor("s4v", [128, OSH // 128, NG], BF,
                          kind="ExternalInput")
    s4_o = nc.dram_tensor("s4o", [128, c.D // 128, NG], BF,
                          kind="ExternalInput")
    # rope tables; partition = s % 128, broadcast over local heads
    cos4_d = nc.dram_tensor("cos4", [128, c.S // 128, 128], FP16,
                            kind="ExternalInput")
    sins4_d = nc.dram_tensor("sins4", [128, c.S // 128, 128], FP16,
                             kind="ExternalInput")
    maskl_d = nc.dram_tensor("maskl", [128, 128], BF, kind="ExternalInput")
    ident_d = nc.dram_tensor("ident", [128, 128], BF, kind="ExternalInput")
    ones_d = nc.dram_tensor("ones", [128, 1], FP16, kind="ExternalInput")
    ebias_d = nc.dram_tensor("ebias", [128, 1], F32, kind="ExternalInput")
    out_d = nc.dram_tensor("out", [c.TPC, c.D], BF, kind="ExternalOutput")

    # collective bounce buffers, one AllToAll per batch
    a2a_in = [nc.dram_tensor(f"a2a_in{b}", [c.NCORES, c.C_SHARD, c.SPC], BF)
              for b in range(c.B)]
    a2a_out = [nc.dram_tensor(f"a2a_out{b}", [c.NCORES, c.C_SHARD, c.SPC], BF)
               for b in range(c.B)]
    # all wo panels, dequantized+transposed, staged via DRAM in phase 1
    wto_d = nc.dram_tensor("wto", [c.D // 512, 128, c.NGP, 512], BF)

    inv_sqrt_d = 1.0 / math.sqrt(128.0)

    # staging plan: 32 wo blocks spread over b1-b3 ts slots
    stage_plan = {}
    nxt = 0
    for b in (1, 2, 3):
        for ts in range(8):
            n = 1 if b == 1 else (2 if ts % 2 == 0 else 1)
            stage_plan[(b, ts)] = list(range(nxt, nxt + n))
            nxt += n
    assert nxt == 32, nxt

    with tile.TileContext(nc) as tc:
        with tc.tile_pool(name="const", bufs=1) as const, \
             tc.tile_pool(name="kqv", bufs=1) as kqvp, \
             tc.tile_pool(name="attn", bufs=2) as attnp, \
             tc.tile_pool(name="scps", bufs=3, space="PSUM") as scps, \
             tc.tile_pool(name="atps", bufs=2, space="PSUM") as atps:

            # ---------- dequant helper ----------
            def dequant_block(pool, q8_t, s4_t, ob, sink, eng0, eng1):
                """Dequantize one 128-oc block: q8 [128, D] i8 times
                per-group scales -> two [128, NGP/2, 128] bf16 slabs in
                natural (partition = oc) layout, handed to sink(hb, comb).
                The two halves run on eng0 / eng1."""
                q8t = pool.tile([128, c.D], I8, tag="q8", bufs=2)
                nc.sync.dma_start(q8t[:], q8_t[ob * 128:(ob + 1) * 128, :])
                q3 = q8t[:].rearrange("o (g f) -> o g f", f=64)
                half = NG // 2  # 32 groups per half-block
                for hb, eng in ((0, eng0), (1, eng1)):
                    comb = pool.tile([128, half, 64], BF, tag="comb", bufs=2)
                    eng.tensor_tensor(
                        out=comb[:],
                        in0=q3[:, hb * half:(hb + 1) * half, :],
                        in1=s4_t[:, ob, hb * half:(hb + 1) * half][:, :, None]
                        .to_broadcast([128, half, 64]),
                        op=AOP.mult)
                    sink(hb, comb)

            def dequant_to_wt(pool, wt, q8_t, s4_t, nob):
                for ob in range(nob):
                    def sink(hb, comb, ob=ob):
                        g0 = hb * (ngp // 2)
                        nc.sync.dma_start(
                            out=wt[:, g0:g0 + ngp // 2,
                                   ob * 128:(ob + 1) * 128],
                            in_=comb[:].rearrange("o (gp t) f -> o gp (t f)",
                                                  t=2),
                            transpose=True)
                    dequant_block(pool, q8_t, s4_t, ob, sink,
                                  nc.vector, nc.gpsimd)

            # ======= phase 1: QKV projections + attention =======
            with tc.tile_pool(name="wt", bufs=1) as wtp, \
                 tc.tile_pool(name="xt", bufs=3) as xtp, \
                 tc.tile_pool(name="dq1", bufs=2) as dq1, \
                 tc.tile_pool(name="rp", bufs=2) as rpp, \
                 tc.tile_pool(name="ppsum", bufs=1, space="PSUM") as ppsum:

                # startup order: wq dequant inputs first, then the x tiles
                # needed immediately, then the remaining constants.
                s4q = const.tile([128, OSH // 128, NG], BF, tag="s4q")
                nc.sync.dma_start(s4q[:], s4_q[:])
                wt_q = wtp.tile([128, c.NGP, OSH], BF, tag="wt_q")
                dequant_to_wt(dq1, wt_q, q8_q, s4q, OSH // 128)

                def load_xt(b, ts):
                    xt = xtp.tile([128, c.NGP, 128], BF, tag="xt")
                    nc.sync.dma_start(xt[:], x4_d.ap()[:, b * 8 + ts, :, :])
                    return xt

                xts0 = [load_xt(0, ts) for ts in range(3)]

                cos4 = const.tile([128, c.S // 128, 128], FP16, tag="cos4")
                nc.sync.dma_start(cos4[:], cos4_d[:])
                sins4 = const.tile([128, c.S // 128, 128], FP16, tag="sin4")
                nc.sync.dma_start(sins4[:], sins4_d[:])
                maskl = const.tile([128, 128], BF, tag="maskl")
                nc.sync.dma_start(maskl[:], maskl_d[:])
                ident = const.tile([128, 128], BF, tag="ident")
                nc.sync.dma_start(ident[:], ident_d[:])
                ones = const.tile([128, 1], FP16, tag="ones")
                nc.sync.dma_start(ones[:], ones_d[:])
                ebias = const.tile([128, 1], F32, tag="ebias")
                nc.sync.dma_start(ebias[:], ebias_d[:])

                s4k = const.tile([128, OSH // 128, NG], BF, tag="s4k")
                nc.sync.dma_start(s4k[:], s4_k[:])
                wt_k = wtp.tile([128, c.NGP, OSH], BF, tag="wt_k")
                dequant_to_wt(dq1, wt_k, q8_k, s4k, OSH // 128)
                s4v = const.tile([128, OSH // 128, NG], BF, tag="s4v")
                nc.sync.dma_start(s4v[:], s4_v[:])
                wt_v = wtp.tile([128, c.NGP, OSH], BF, tag="wt_v")
                dequant_to_wt(dq1, wt_v, q8_v, s4v, OSH // 128)
                s4o = const.tile([128, c.D // 128, NG], BF, tag="s4o")
                nc.sync.dma_start(s4o[:], s4_o[:])

                def proj_pass(xt, wt_m, mat):
                    ps = ppsum.tile([128, OSH], F32, tag="pp", bufs=2,
                                    name="ps_" + mat)
                    for gp in range(c.NGP):
                        nc.tensor.matmul(
                            ps[:], lhsT=xt[:, gp, :], rhs=wt_m[:, gp, :],
                            start=(gp == 0), stop=(gp == c.NGP - 1))
                    return ps

                def evac_rope(ts, mat, ps, kt_b, qt_b, v_b):
                    st0 = ts * 128
                    if mat == "v":
                        nc.scalar.copy(out=v_b[:, ts, :], in_=ps[:])
                        return
                    # evacuate the PSUM bank fast (ACT), rope from SBUF
                    ev = rpp.tile([128, c.C_SHARD], FP16, tag="ev" + mat,
                                  bufs=2)
                    nc.scalar.copy(out=ev[:], in_=ps[:])
                    roped = rpp.tile([128, c.C_SHARD], FP16, tag="ro" + mat,
                                     bufs=2)
                    tmp = rpp.tile([128, c.C_SHARD], FP16, tag="tm" + mat,
                                   bufs=1)
                    p3 = ev[:].rearrange("p (h d) -> p h d", d=128)
                    t3 = tmp[:].rearrange("p (h d) -> p h d", d=128)
                    nc.vector.tensor_tensor(
                        out=t3[:, :, 0:64], in0=p3[:, :, 64:128],
                        in1=sins4[:, ts, None, 0:64]
                        .to_broadcast([128, c.H_LOC, 64]),
                        op=AOP.mult)
                    nc.vector.tensor_tensor(
                        out=t3[:, :, 64:128], in0=p3[:, :, 0:64],
                        in1=sins4[:, ts, None, 64:128]
                        .to_broadcast([128, c.H_LOC, 64]),
                        op=AOP.mult)
                    r3 = roped[:].rearrange("p (h d) -> p h d", d=128)
                    nc.vector.tensor_tensor(
                        out=r3[:], in0=p3[:],
                        in1=cos4[:, ts, None, :]
                        .to_broadcast([128, c.H_LOC, 128]),
                        op=AOP.mult)
                    nc.vector.tensor_tensor(
                        out=roped[:], in0=roped[:], in1=tmp[:], op=AOP.add)
                    dst = qt_b if mat == "q" else kt_b
                    nc.sync.dma_start(
                        out=dst[:, :, st0:st0 + 128], in_=roped[:],
                        transpose=True)

                # ---- wo staging (all panels -> DRAM) in phase-1 slack ----
                def stage_wo_block(ob):
                    def sink(hb, comb):
                        g0 = hb * (ngp // 2)
                        wtmp = dq1.tile([128, ngp // 2, 128], BF, tag="wtmp",
                                        bufs=1)
                        nc.sync.dma_start(
                            out=wtmp[:],
                            in_=comb[:].rearrange("o (gp t) f -> o gp (t f)",
                                                  t=2),
                            transpose=True)
                        nc.sync.dma_start(
                            out=wto_d.ap()[ob // 4][:, g0:g0 + ngp // 2,
                                                    (ob % 4) * 128:
                                                    (ob % 4 + 1) * 128],
                            in_=wtmp[:])
                    dequant_block(dq1, q8_o, s4o, ob, sink,
                                  nc.vector, nc.gpsimd)

                # ---------- attention ----------
                def attention_mm(b, qc, h0, h1, kt_b, qt_b, v_b):
                    """Matmul phase of two heads' chains for one q-chunk,
                    interleaved per k-tile; AV trails scores by one tile.
                    The at accumulators are evacuated to SBUF immediately;
                    the softmax tail runs later (attention_tail)."""
                    q0 = qc * 512
                    K = 4 if qc == 0 else 8
                    hs = (h0, h1)
                    ats = [atps.tile([128, c.QCH], F32, tag="at",
                                     name="at") for _ in range(2)]
                    ptrees = [attnp.tile([128, c.QCH], FP16, tag="ptree",
                                         bufs=4, name="ptree")
                              for _ in range(2)]
                    pts = {}
                    offs = {}

                    def scores(ci, ki):
                        h = hs[ci]
                        off = max(0, 128 * ki - q0)
                        offs[(ci, ki)] = off
                        diag = 128 * ki >= q0
                        sp = scps.tile([128, c.QCH], F32, tag="sc",
                                       name="sc")
                        nc.tensor.matmul(
                            sp[:, off:],
                            lhsT=kt_b[:, h, ki * 128:(ki + 1) * 128],
                            rhs=qt_b[:, h, q0 + off:q0 + c.QCH],
                            start=True, stop=not diag)
                        if diag:
                            nc.tensor.matmul(
                                sp[:, off:off + 128], lhsT=maskl[:],
                                rhs=ident[:], start=False, stop=True)
                        pt = attnp.tile([128, c.QCH], FP16, tag="pt",
                                        bufs=4, name="pt")
                        nc.scalar.activation(
                            out=pt[:, off:], in_=sp[:, off:], func=AF.Exp,
                            scale=inv_sqrt_d, bias=ebias[:, 0:1])
                        if ki == 0:
                            nc.vector.tensor_copy(out=ptrees[ci][:],
                                                  in_=pt[:])
                        else:
                            nc.vector.tensor_tensor(
                                out=ptrees[ci][:, off:],
                                in0=ptrees[ci][:, off:],
                                in1=pt[:, off:], op=AOP.add)
                        pts[(ci, ki)] = pt

                    def av(ci, j):
                        off = offs[(ci, j)]
                        nc.tensor.matmul(
                            ats[ci][:, off:],
                            lhsT=v_b[:, j, hs[ci] * 128:(hs[ci] + 1) * 128],
                            rhs=pts[(ci, j)][:, off:],
                            start=(j == 0), stop=(j == K - 1))

                    for st in range(K + 1):
                        if st < K:
                            scores(0, st)
                            scores(1, st)
                        if st >= 1:
                            av(0, st - 1)
                            av(1, st - 1)

                    ass = []
                    for ci in range(2):
                        a_sb = attnp.tile([128, c.QCH], BF, tag="as",
                                          bufs=4, name="a_sb")
                        nc.scalar.copy(out=a_sb[:], in_=ats[ci][:])
                        ass.append(a_sb)
                    return (b, qc, hs, ptrees, ass)

                def attention_tail(ctx):
                    """Deferred softmax tail: z = ones^T ptree (into the
                    freed at banks via the same tag rotation), reciprocal,
                    partition broadcast, ao = at * rz, a2a scatter."""
                    b, qc, hs, ptrees, ass = ctx
                    for ci in range(2):
                        z = atps.tile([128, c.QCH], F32, tag="at", name="z")
                        nc.tensor.matmul(z[0:1, :], lhsT=ones[:, 0:1],
                                         rhs=ptrees[ci][:], start=True,
                                         stop=True)
                        rz = attnp.tile([1, c.QCH], F32, tag="rz", bufs=2)
                        nc.vector.reciprocal_approx_fast(rz[:], z[0:1, :])
                        rzb = attnp.tile([128, c.QCH], F32, tag="rzb",
                                         bufs=2)
                        nc.gpsimd.partition_broadcast(rzb[:], rz[:])
                        ao = attnp.tile([128, c.QCH], BF, tag="ao", bufs=2)
                        nc.vector.tensor_tensor(
                            out=ao[:], in0=ass[ci][:], in1=rzb[:],
                            op=AOP.mult)
                        nc.sync.dma_start(
                            out=a2a_in[b][qc * 4:(qc + 1) * 4,
                                          hs[ci] * 128:(hs[ci] + 1) * 128, :]
                            .rearrange("r c s -> c r s"),
                            in_=ao[:])

                def attention(b, kt_b, qt_b, v_b):
                    # first-half q-chunks first (they only need the first
                    # half of kt/qt/v); each pair's tail is deferred one
                    # pair so its DVE inputs are ready when the PE reaches
                    # the z matmuls
                    ctxs = []
                    for qc, h0, h1 in ((0, 0, 1), (0, 2, 3),
                                       (1, 0, 1), (1, 2, 3)):
                        ctxs.append(attention_mm(b, qc, h0, h1,
                                                 kt_b, qt_b, v_b))
                        if len(ctxs) >= 2:
                            attention_tail(ctxs[-2])
                    attention_tail(ctxs[-1])

                def do_a2a(b):
                    nc.gpsimd.collective_compute(
                        "AllToAll", AOP.bypass,
                        replica_groups=[list(range(c.NCORES))],
                        ins=[a2a_in[b].ap().opt()],
                        outs=[a2a_out[b].ap().opt()],
                    )

                # ---------- phase-1 main loop ----------
                kqv = {}
                for b in range(c.B):
                    kt_b = kqvp.tile([128, c.H_LOC, c.S], FP16, tag="kt_b")
                    qt_b = kqvp.tile([128, c.H_LOC, c.S], FP16, tag="qt_b")
                    v_b = kqvp.tile([128, c.S // 128, c.C_SHARD], FP16,
                                    tag="v_b")
                    kqv[b] = (kt_b, qt_b, v_b)

                    if b == 0:
                        # q-passes first: only wt_q is needed to start
                        for mat, wt_m in (("q", wt_q), ("k", wt_k),
                                          ("v", wt_v)):
                            for ts in range(3):
                                ps = proj_pass(xts0[ts], wt_m, mat)
                                evac_rope(ts, mat, ps, kt_b, qt_b, v_b)
                        ts_range = range(3, c.S // 128)
                    else:
                        ts_range = range(c.S // 128)

                    for ts in ts_range:
                        xt = load_xt(b, ts)
                        for mat, wt_m in (("q", wt_q), ("k", wt_k),
                                          ("v", wt_v)):
                            ps = proj_pass(xt, wt_m, mat)
                            evac_rope(ts, mat, ps, kt_b, qt_b, v_b)
                        # staged after the ropes so the DVE half doesn't
                        # head-of-line delay them
                        for ob in stage_plan.get((b, ts), ()):
                            stage_wo_block(ob)

                    if b < c.B - 1:
                        attention(b, kt_b, qt_b, v_b)
                        do_a2a(b)

            # ======= phase 2: batch-3 attention + output projection =======
            with tc.tile_pool(name="gath", bufs=1) as gathp, \
                 tc.tile_pool(name="wop", bufs=1) as wopp, \
                 tc.tile_pool(name="osb", bufs=3) as osbp, \
                 tc.tile_pool(name="wpsum", bufs=3, space="PSUM") as wpsum:

                gath = gathp.tile([128, c.NGP, c.TPC], BF)

                def gather_b(b):
                    nc.sync.dma_start(
                        gath[:, :, b * c.SPC:(b + 1) * c.SPC],
                        a2a_out[b].ap().rearrange("s (g p) t -> p (s g) t",
                                                  p=128))

                def load_panel(oc):
                    panel = wopp.tile([128, c.NGP, 512], BF, tag="wop",
                                      bufs=3, name="panel")
                    nc.sync.dma_start(panel[:], wto_d.ap()[oc])
                    return panel

                p0 = load_panel(0)
                p1 = load_panel(1)
                for b in range(c.B - 1):
                    gather_b(b)
                p2 = load_panel(2)

                def wo_store(ops, oc, tb):
                    osb = osbp.tile([128, 512], BF, tag="osb", bufs=3,
                                    name="osb")
                    nc.scalar.copy(out=osb[:], in_=ops[:])
                    nc.sync.dma_start(
                        out=out_d[tb * 128:(tb + 1) * 128,
                                  oc * 512:(oc + 1) * 512],
                        in_=osb[:])

                def wo_pass(pa, pb, oca, ocb, tb):
                    opsa = wpsum.tile([128, 512], F32, tag="wo", name="wo")
                    opsb = wpsum.tile([128, 512], F32, tag="wo", name="wo")
                    for ct in range(c.NGP):
                        lhsT = gath[:, ct, tb * 128:(tb + 1) * 128]
                        nc.tensor.matmul(opsa[:], lhsT=lhsT, rhs=pa[:, ct, :],
                                         start=(ct == 0),
                                         stop=(ct == c.NGP - 1))
                        nc.tensor.matmul(opsb[:], lhsT=lhsT, rhs=pb[:, ct, :],
                                         start=(ct == 0),
                                         stop=(ct == c.NGP - 1))
                    wo_store(opsa, oca, tb)
                    wo_store(opsb, ocb, tb)

                def wo_single(pa, oca, tb):
                    ops = wpsum.tile([128, 512], F32, tag="wo", name="wo")
                    for ct in range(c.NGP):
                        nc.tensor.matmul(
                            ops[:], lhsT=gath[:, ct, tb * 128:(tb + 1) * 128],
                            rhs=pa[:, ct, :], start=(ct == 0),
                            stop=(ct == c.NGP - 1))
                    wo_store(ops, oca, tb)

                # batch-3 attention interleaved with early wo passes;
                # tails deferred one pair as in phase 1
                kt3, qt3, v3 = kqv[c.B - 1]
                ctxA1 = attention_mm(c.B - 1, 0, 0, 1, kt3, qt3, v3)
                ctxA2 = attention_mm(c.B - 1, 0, 2, 3, kt3, qt3, v3)
                attention_tail(ctxA1)
                wo_pass(p0, p1, 0, 1, 0)
                ctxB1 = attention_mm(c.B - 1, 1, 0, 1, kt3, qt3, v3)
                attention_tail(ctxA2)
                wo_pass(p0, p1, 0, 1, 1)
                ctxB2 = attention_mm(c.B - 1, 1, 2, 3, kt3, qt3, v3)
                attention_tail(ctxB1)
                attention_tail(ctxB2)
                do_a2a(c.B - 1)
                gather_b(c.B - 1)
                wo_single(p2, 2, 0)           # fills the collective window
                wo_pass(p0, p1, 0, 1, 2)
                wo_pass(p0, p1, 0, 1, 3)      # needs gather-3

                # rolling panel pipeline: singles cover each new panel's
                # load latency
                p3 = load_panel(3)            # reuses p0's slot (now free)
                p4 = load_panel(4)            # reuses p1's slot
                wo_single(p2, 2, 1)
                wo_single(p2, 2, 2)
                wo_single(p2, 2, 3)
                p5 = load_panel(5)            # reuses p2's slot
                for tb in range(4):
                    wo_pass(p3, p4, 3, 4, tb)
                p6 = load_panel(6)            # reuses p3's slot
                p7 = load_panel(7)            # reuses p4's slot
                wo_single(p5, 5, 0)
                for tb in (1, 2, 3):
                    wo_pass(p5, p6, 5, 6, tb)
                wo_single(p6, 6, 0)
                wo_single(p7, 7, 0)
                wo_single(p7, 7, 1)
                wo_single(p7, 7, 2)
                wo_single(p7, 7, 3)

    nc.compile()
    return nc


# ---------------- host-side input prep ----------------

def prep_core_inputs(cfg: Cfg, x, cos_half, sin_half, mask,
                     wq_w, wq_s, wk_w, wk_s, wv_w, wv_s, wo_w, wo_s):
    """Build in_maps (list of dicts, one per core) from full inputs."""
    import ml_dtypes
    c = cfg
    bf16 = ml_dtypes.bfloat16
    HD2 = 64
    OSH = c.C_SHARD
    ngp = c.NGP

    # x pre-tiled: x4[p, b*8+ts, g, t] = x[b, ts*128+t, g*128+p]
    xr = np.asarray(x).reshape(c.B, c.S // 128, 128, ngp, 128)
    x4 = np.ascontiguousarray(xr.transpose(4, 0, 1, 3, 2)).reshape(
        128, c.T // 128, ngp, 128)

    # rope tables [128, S//128, 128] fp16 (bf16-rounded values)
    ch = np.asarray(cos_half, np.float32)  # [S, 64]
    sh = np.asarray(sin_half, np.float32)
    cos = np.concatenate([ch, ch], axis=1).astype(bf16).astype(np.float32)
    sin = np.concatenate([sh, sh], axis=1).astype(bf16).astype(np.float32)
    sins = sin.copy()
    sins[:, :HD2] = -sin[:, :HD2]
    cos4 = np.ascontiguousarray(
        cos.reshape(c.S // 128, 128, 128).transpose(1, 0, 2)).astype(
        np.float16)
    sins4 = np.ascontiguousarray(
        sins.reshape(c.S // 128, 128, 128).transpose(1, 0, 2)).astype(
        np.float16)

    # causal-mask lhsT for the diagonal-block mask matmul
    m = np.asarray(mask, np.float32)[:128, :128]
    maskl = np.maximum(m, -1e30).astype(bf16)
    ident = np.eye(128, dtype=np.float32).astype(bf16)
    ones = np.ones((128, 1), np.float16)
    ebias = np.full((128, 1), -4.0, np.float32)

    def unpack_q8(pw, n_oc):
        """Packed Q4_0 rows -> int8 [n_oc, D] in c order."""
        w_ = np.asarray(pw).reshape(n_oc, ngp, 64)
        msb = (w_ >> 4).astype(np.int8)
        lsb = (((w_ & 15) ^ 8) - 8).astype(np.int8)
        q8 = np.stack([msb, lsb], axis=2)  # [oc, r, 2, 64]
        return np.ascontiguousarray(q8.reshape(n_oc, ngp * 128))

    def scale4(ps, n_oc):
        """Scales -> [128, n_oc//128, 2*ngp] (p, ob, g)."""
        a = np.asarray(ps).reshape(n_oc, 2 * ngp)
        return np.ascontiguousarray(
            a.reshape(n_oc // 128, 128, 2 * ngp).transpose(1, 0, 2))

    in_maps = []
    for core in range(c.NCORES):
        RPO = ngp
        r0 = core * OSH * RPO
        g0 = core * OSH * 2 * RPO
        in_maps.append({
            "x4": x4,
            "q8q": unpack_q8(np.asarray(wq_w)[r0:r0 + OSH * RPO], OSH),
            "q8k": unpack_q8(np.asarray(wk_w)[r0:r0 + OSH * RPO], OSH),
            "q8v": unpack_q8(np.asarray(wv_w)[r0:r0 + OSH * RPO], OSH),
            "q8o": unpack_q8(np.asarray(wo_w), c.D),
            "s4q": scale4(np.asarray(wq_s)[g0:g0 + OSH * 2 * RPO], OSH),
            "s4k": scale4(np.asarray(wk_s)[g0:g0 + OSH * 2 * RPO], OSH),
            "s4v": scale4(np.asarray(wv_s)[g0:g0 + OSH * 2 * RPO], OSH),
            "s4o": scale4(np.asarray(wo_s), c.D),
            "cos4": cos4,
            "sins4": sins4,
            "maskl": maskl,
            "ident": ident,
            "ones": ones,
            "ebias": ebias,
        })
    return in_maps


def unshard_output(cfg: Cfg, results):
    """results: list per core of {"out": [TPC, D]}; core r's rows are
    (b, s1) with seq slice [128r, 128(r+1)) of every batch."""
    c = cfg
    full = np.empty((c.B, c.S, c.D), dtype=np.asarray(results[0]["out"]).dtype)
    for r in range(c.NCORES):
        o = np.asarray(results[r]["out"]).reshape(c.B, c.SPC, c.D)
        full[:, r * c.SPC:(r + 1) * c.SPC, :] = o
    return full


# ======================================================================
# Self-contained kernel entry point.
# ======================================================================

_CACHE = {}


def _get_program(cfg):
    key = (cfg.B, cfg.S, cfg.D, cfg.NCORES, cfg.SCH, cfg.QCH)
    if key not in _CACHE:
        _CACHE[key] = build_program(cfg)
    return _CACHE[key]


def kernel(x, start_pos=0, cos_half=None, sin_half=None, mask=None,
           wq_w=None, wq_s=None, wk_w=None, wk_s=None,
           wv_w=None, wv_s=None, wo_w=None, wo_s=None,
           cache_k_w=None, cache_k_s=None, cache_v_w=None, cache_v_s=None,
           **_unused):
    from concourse.bass_utils import run_bass_kernel_spmd

    assert int(start_pos) == 0, "kernel specialised for start_pos == 0"
    x = np.asarray(x)
    B, S, D = x.shape
    cfg = Cfg(B=B, S=S, D=D, NCORES=8, SCH=512, QCH=512)
    # start_pos==0 with S==MAX_S, B==MAX_B: the quantized KV cache is fully
    # overwritten before use, so cache_* inputs cannot affect the output.
    in_maps = prep_core_inputs(cfg, x, cos_half, sin_half, mask,
                               wq_w, wq_s, wk_w, wk_s, wv_w, wv_s,
                               wo_w, wo_s)
    nc = _get_program(cfg)
    res = run_bass_kernel_spmd(nc, in_maps, core_ids=list(range(cfg.NCORES)))
    out = unshard_output(cfg, res.results)
    import ml_dtypes
    return out.astype(ml_dtypes.bfloat16, copy=False)


# revision 39
# speedup vs baseline: 1.0242x; 1.0242x over previous
"""Trainium2 (Bass/Tile) kernel for quantized multi-head attention.

Distributed across 8 NeuronCores: tensor-parallel over heads for the
Q4_0-dequant + QKV projections + RoPE + causal attention, one small
AllToAll per batch (overlapped with later batches), then a
token-parallel output projection.

Scheduling notes:
 - dequant is one broadcast-multiply per half-block (host ships nibbles
   pre-widened to int8), halves alternating DVE / GpSimd, then an xbar
   transpose into [c, oc] layout.
 - batch 0 runs q-passes for ts0-3 first so matmuls start as soon as
   wt_q is ready while wt_k / wt_v still dequantize.
 - all eight wo panels are dequantized into a DRAM staging buffer
   during phase-1 slack; phase 2 only re-loads them (no compute on the
   critical path).
 - attention interleaves head pairs per k-tile with the AV matmuls
   trailing one tile (hides the Exp latency); the first-half q-chunks
   run first because they only need the first half of the per-batch
   Q/K/V tiles.  The causal diagonal mask and the softmax denominator
   are computed with tiny matmuls (mask @ I accumulate, ones^T @ ptree).
 - wo matmuls on staged panels interleave into batch-3's attention so
   the final AllToAll is off the critical path.
"""

import math
from dataclasses import dataclass

import numpy as np

import concourse.bass as bass
import concourse.tile as tile
from concourse import bacc, mybir, bass_isa

BF = mybir.dt.bfloat16
FP16 = mybir.dt.float16
F32 = mybir.dt.float32
I8 = mybir.dt.int8
AOP = mybir.AluOpType
AF = mybir.ActivationFunctionType


@dataclass
class Cfg:
    B: int = 4
    S: int = 1024
    D: int = 4096
    NCORES: int = 8
    SCH: int = 512   # kept for test.py compat (unused)
    QCH: int = 512   # attention q-chunk

    @property
    def T(self):
        return self.B * self.S

    @property
    def H(self):
        return self.D // 128  # total heads (head_dim 128)

    @property
    def H_LOC(self):
        return self.H // self.NCORES

    @property
    def C_SHARD(self):
        return self.H_LOC * 128  # local channels

    @property
    def SPC(self):
        return self.S // self.NCORES  # seq slice per core per batch (128)

    @property
    def TPC(self):
        return self.B * self.SPC  # tokens per core (output slice)

    @property
    def NGP(self):
        return self.D // 128  # contraction k-tiles / group-pairs per row


def build_program(cfg: Cfg):
    """Build the per-core Bass program. Returns compiled nc."""
    c = cfg
    assert c.S % c.QCH == 0 and c.QCH <= 512
    assert c.S % (128 * c.NCORES) == 0

    import concourse.tile_utils as tile_utils
    tile_utils.max_sbuf_usage = 208 * 1024

    nc = bacc.Bacc("TRN2", target_bir_lowering=False, debug=False,
                   num_devices=c.NCORES)

    OSH = c.C_SHARD      # qkv weight shard out-channels per core
    NG = 2 * c.NGP       # scale groups (of 64) per out-channel row
    ngp = c.NGP

    # ---- external I/O ----
    # x pre-tiled on host: [p, b*8+ts, g, t]
    x4_d = nc.dram_tensor("x4", [128, c.T // 128, ngp, 128], BF,
                          kind="ExternalInput")
    q8_q = nc.dram_tensor("q8q", [OSH, c.D], I8, kind="ExternalInput")
    q8_k = nc.dram_tensor("q8k", [OSH, c.D], I8, kind="ExternalInput")
    q8_v = nc.dram_tensor("q8v", [OSH, c.D], I8, kind="ExternalInput")
    q8_o = nc.dram_tensor("q8o", [c.D, c.D], I8, kind="ExternalInput")
    s4_q = nc.dram_tensor("s4q", [128, OSH // 128, NG], BF,
                          kind="ExternalInput")
    s4_k = nc.dram_tensor("s4k", [128, OSH // 128, NG], BF,
                          kind="ExternalInput")
    s4_v = nc.dram_tensor("s4v", [128, OSH // 128, NG], BF,
                          kind="ExternalInput")
    s4_o = nc.dram_tensor("s4o", [128, c.D // 128, NG], BF,
                          kind="ExternalInput")
    # rope tables; partition = s % 128, broadcast over local heads
    cos4_d = nc.dram_tensor("cos4", [128, c.S // 128, 128], FP16,
                            kind="ExternalInput")
    sins4_d = nc.dram_tensor("sins4", [128, c.S // 128, 128], FP16,
                             kind="ExternalInput")
    maskl_d = nc.dram_tensor("maskl", [128, 128], BF, kind="ExternalInput")
    ident_d = nc.dram_tensor("ident", [128, 128], BF, kind="ExternalInput")
    ones_d = nc.dram_tensor("ones", [128, 1], FP16, kind="ExternalInput")
    ebias_d = nc.dram_tensor("ebias", [128, 1], F32, kind="ExternalInput")
    out_d = nc.dram_tensor("out", [c.TPC, c.D], BF, kind="ExternalOutput")

    # collective bounce buffers, one AllToAll per batch
    a2a_in = [nc.dram_tensor(f"a2a_in{b}", [c.NCORES, c.C_SHARD, c.SPC], BF)
              for b in range(c.B)]
    a2a_out = [nc.dram_tensor(f"a2a_out{b}", [c.NCORES, c.C_SHARD, c.SPC], BF)
               for b in range(c.B)]
    # all wo panels, dequantized+transposed, staged via DRAM in phase 1
    wto_d = nc.dram_tensor("wto", [c.D // 512, 128, c.NGP, 512], BF)

    inv_sqrt_d = 1.0 / math.sqrt(128.0)

    # staging plan: 32 wo blocks spread over b1-b3 ts slots; ts6-7 are
    # kept clear so the batch-end rope->transpose chain and attention
    # never queue behind staging work
    stage_plan = {}
    nxt = 0
    for b in (1, 2, 3):
        for ts in range(6):
            n = 2 if (b > 1 or ts in (0, 5)) else 1
            stage_plan[(b, ts)] = list(range(nxt, nxt + n))
            nxt += n
    assert nxt == 32, nxt

    with tile.TileContext(nc) as tc:
        with tc.tile_pool(name="const", bufs=1) as const, \
             tc.tile_pool(name="kqv", bufs=1) as kqvp, \
             tc.tile_pool(name="attn", bufs=2) as attnp, \
             tc.tile_pool(name="scps", bufs=3, space="PSUM") as scps, \
             tc.tile_pool(name="atps", bufs=2, space="PSUM") as atps:

            # ---------- dequant helper ----------
            def dequant_block(pool, q8_t, s4_t, ob, sink, eng0, eng1):
                """Dequantize one 128-oc block: q8 [128, D] i8 times
                per-group scales -> two [128, NGP/2, 128] bf16 slabs in
                natural (partition = oc) layout, handed to sink(hb, comb).
                The two halves run on eng0 / eng1."""
                q8t = pool.tile([128, c.D], I8, tag="q8", bufs=2)
                nc.sync.dma_start(q8t[:], q8_t[ob * 128:(ob + 1) * 128, :])
                q3 = q8t[:].rearrange("o (g f) -> o g f", f=64)
                half = NG // 2  # 32 groups per half-block
                for hb, eng in ((0, eng0), (1, eng1)):
                    comb = pool.tile([128, half, 64], BF, tag="comb", bufs=2)
                    eng.tensor_tensor(
                        out=comb[:],
                        in0=q3[:, hb * half:(hb + 1) * half, :],
                        in1=s4_t[:, ob, hb * half:(hb + 1) * half][:, :, None]
                        .to_broadcast([128, half, 64]),
                        op=AOP.mult)
                    sink(hb, comb)

            def dequant_to_wt(pool, wt, q8_t, s4_t, nob):
                for ob in range(nob):
                    def sink(hb, comb, ob=ob):
                        g0 = hb * (ngp // 2)
                        nc.sync.dma_start(
                            out=wt[:, g0:g0 + ngp // 2,
                                   ob * 128:(ob + 1) * 128],
                            in_=comb[:].rearrange("o (gp t) f -> o gp (t f)",
                                                  t=2),
                            transpose=True)
                    dequant_block(pool, q8_t, s4_t, ob, sink,
                                  nc.vector, nc.gpsimd)

            # ======= phase 1: QKV projections + attention =======
            with tc.tile_pool(name="wt", bufs=1) as wtp, \
                 tc.tile_pool(name="xt", bufs=3) as xtp, \
                 tc.tile_pool(name="dq1", bufs=2) as dq1, \
                 tc.tile_pool(name="rp", bufs=2) as rpp, \
                 tc.tile_pool(name="ppsum", bufs=1, space="PSUM") as ppsum:

                # startup order: wq dequant inputs first, then the x tiles
                # needed immediately, then the remaining constants.
                s4q = const.tile([128, OSH // 128, NG], BF, tag="s4q")
                nc.sync.dma_start(s4q[:], s4_q[:])
                wt_q = wtp.tile([128, c.NGP, OSH], BF, tag="wt_q")
                dequant_to_wt(dq1, wt_q, q8_q, s4q, OSH // 128)

                def load_xt(b, ts):
                    xt = xtp.tile([128, c.NGP, 128], BF, tag="xt")
                    nc.sync.dma_start(xt[:], x4_d.ap()[:, b * 8 + ts, :, :])
                    return xt

                xts0 = [load_xt(0, ts) for ts in range(3)]

                cos4 = const.tile([128, c.S // 128, 128], FP16, tag="cos4")
                nc.sync.dma_start(cos4[:], cos4_d[:])
                sins4 = const.tile([128, c.S // 128, 128], FP16, tag="sin4")
                nc.sync.dma_start(sins4[:], sins4_d[:])
                maskl = const.tile([128, 128], BF, tag="maskl")
                nc.sync.dma_start(maskl[:], maskl_d[:])
                ident = const.tile([128, 128], BF, tag="ident")
                nc.sync.dma_start(ident[:], ident_d[:])
                ones = const.tile([128, 1], FP16, tag="ones")
                nc.sync.dma_start(ones[:], ones_d[:])
                ebias = const.tile([128, 1], F32, tag="ebias")
                nc.sync.dma_start(ebias[:], ebias_d[:])

                s4k = const.tile([128, OSH // 128, NG], BF, tag="s4k")
                nc.sync.dma_start(s4k[:], s4_k[:])
                wt_k = wtp.tile([128, c.NGP, OSH], BF, tag="wt_k")
                dequant_to_wt(dq1, wt_k, q8_k, s4k, OSH // 128)
                s4v = const.tile([128, OSH // 128, NG], BF, tag="s4v")
                nc.sync.dma_start(s4v[:], s4_v[:])
                wt_v = wtp.tile([128, c.NGP, OSH], BF, tag="wt_v")
                dequant_to_wt(dq1, wt_v, q8_v, s4v, OSH // 128)
                s4o = const.tile([128, c.D // 128, NG], BF, tag="s4o")
                nc.sync.dma_start(s4o[:], s4_o[:])

                def proj_pass(xt, wt_m, mat):
                    ps = ppsum.tile([128, OSH], F32, tag="pp", bufs=2,
                                    name="ps_" + mat)
                    for gp in range(c.NGP):
                        nc.tensor.matmul(
                            ps[:], lhsT=xt[:, gp, :], rhs=wt_m[:, gp, :],
                            start=(gp == 0), stop=(gp == c.NGP - 1))
                    return ps

                def evac_rope(ts, mat, ps, kt_b, qt_b, v_b):
                    st0 = ts * 128
                    if mat == "v":
                        nc.scalar.copy(out=v_b[:, ts, :], in_=ps[:])
                        return
                    # evacuate the PSUM bank fast (ACT), rope from SBUF
                    ev = rpp.tile([128, c.C_SHARD], FP16, tag="ev" + mat,
                                  bufs=2)
                    nc.scalar.copy(out=ev[:], in_=ps[:])
                    roped = rpp.tile([128, c.C_SHARD], FP16, tag="ro" + mat,
                                     bufs=2)
                    tmp = rpp.tile([128, c.C_SHARD], FP16, tag="tm" + mat,
                                   bufs=1)
                    p3 = ev[:].rearrange("p (h d) -> p h d", d=128)
                    t3 = tmp[:].rearrange("p (h d) -> p h d", d=128)
                    nc.vector.tensor_tensor(
                        out=t3[:, :, 0:64], in0=p3[:, :, 64:128],
                        in1=sins4[:, ts, None, 0:64]
                        .to_broadcast([128, c.H_LOC, 64]),
                        op=AOP.mult)
                    nc.vector.tensor_tensor(
                        out=t3[:, :, 64:128], in0=p3[:, :, 0:64],
                        in1=sins4[:, ts, None, 64:128]
                        .to_broadcast([128, c.H_LOC, 64]),
                        op=AOP.mult)
                    r3 = roped[:].rearrange("p (h d) -> p h d", d=128)
                    nc.vector.tensor_tensor(
                        out=r3[:], in0=p3[:],
                        in1=cos4[:, ts, None, :]
                        .to_broadcast([128, c.H_LOC, 128]),
                        op=AOP.mult)
                    nc.vector.tensor_tensor(
                        out=roped[:], in0=roped[:], in1=tmp[:], op=AOP.add)
                    dst = qt_b if mat == "q" else kt_b
                    nc.sync.dma_start(
                        out=dst[:, :, st0:st0 + 128], in_=roped[:],
                        transpose=True)

                # ---- wo staging (all panels -> DRAM) in phase-1 slack ----
                def stage_wo_block(ob):
                    def sink(hb, comb):
                        g0 = hb * (ngp // 2)
                        wtmp = dq1.tile([128, ngp // 2, 128], BF, tag="wtmp",
                                        bufs=1)
                        nc.sync.dma_start(
                            out=wtmp[:],
                            in_=comb[:].rearrange("o (gp t) f -> o gp (t f)",
                                                  t=2),
                            transpose=True)
                        nc.sync.dma_start(
                            out=wto_d.ap()[ob // 4][:, g0:g0 + ngp // 2,
                                                    (ob % 4) * 128:
                                                    (ob % 4 + 1) * 128],
                            in_=wtmp[:])
                    dequant_block(dq1, q8_o, s4o, ob, sink,
                                  nc.vector, nc.gpsimd)

                # ---------- attention ----------
                def attention_mm(b, qc, h0, h1, kt_b, qt_b, v_b):
                    """Matmul phase of two heads' chains for one q-chunk,
                    interleaved per k-tile; AV trails scores by one tile.
                    The at accumulators are evacuated to SBUF immediately;
                    the softmax tail runs later (attention_tail)."""
                    q0 = qc * 512
                    K = 4 if qc == 0 else 8
                    hs = (h0, h1)
                    ats = [atps.tile([128, c.QCH], F32, tag="at",
                                     name="at") for _ in range(2)]
                    ptrees = [attnp.tile([128, c.QCH], FP16, tag="ptree",
                                         bufs=4, name="ptree")
                              for _ in range(2)]
                    pts = {}
                    offs = {}

                    def scores(ci, ki):
                        h = hs[ci]
                        off = max(0, 128 * ki - q0)
                        offs[(ci, ki)] = off
                        diag = 128 * ki >= q0
                        sp = scps.tile([128, c.QCH], F32, tag="sc",
                                       name="sc")
                        nc.tensor.matmul(
                            sp[:, off:],
                            lhsT=kt_b[:, h, ki * 128:(ki + 1) * 128],
                            rhs=qt_b[:, h, q0 + off:q0 + c.QCH],
                            start=True, stop=not diag)
                        if diag:
                            nc.tensor.matmul(
                                sp[:, off:off + 128], lhsT=maskl[:],
                                rhs=ident[:], start=False, stop=True)
                        pt = attnp.tile([128, c.QCH], FP16, tag="pt",
                                        bufs=4, name="pt")
                        nc.scalar.activation(
                            out=pt[:, off:], in_=sp[:, off:], func=AF.Exp,
                            scale=inv_sqrt_d, bias=ebias[:, 0:1])
                        if ki == 0:
                            nc.vector.tensor_copy(out=ptrees[ci][:],
                                                  in_=pt[:])
                        else:
                            nc.vector.tensor_tensor(
                                out=ptrees[ci][:, off:],
                                in0=ptrees[ci][:, off:],
                                in1=pt[:, off:], op=AOP.add)
                        pts[(ci, ki)] = pt

                    def av(ci, j):
                        off = offs[(ci, j)]
                        nc.tensor.matmul(
                            ats[ci][:, off:],
                            lhsT=v_b[:, j, hs[ci] * 128:(hs[ci] + 1) * 128],
                            rhs=pts[(ci, j)][:, off:],
                            start=(j == 0), stop=(j == K - 1))

                    for st in range(K + 1):
                        if st < K:
                            scores(0, st)
                            scores(1, st)
                        if st >= 1:
                            av(0, st - 1)
                            av(1, st - 1)

                    ass = []
                    for ci in range(2):
                        a_sb = attnp.tile([128, c.QCH], BF, tag="as",
                                          bufs=4, name="a_sb")
                        nc.scalar.copy(out=a_sb[:], in_=ats[ci][:])
                        ass.append(a_sb)
                    return (b, qc, hs, ptrees, ass)

                def attention_tail(ctx):
                    """Deferred softmax tail: z = ones^T ptree (into the
                    freed at banks via the same tag rotation), reciprocal,
                    partition broadcast, ao = at * rz, a2a scatter."""
                    b, qc, hs, ptrees, ass = ctx
                    for ci in range(2):
                        z = atps.tile([128, c.QCH], F32, tag="at", name="z")
                        nc.tensor.matmul(z[0:1, :], lhsT=ones[:, 0:1],
                                         rhs=ptrees[ci][:], start=True,
                                         stop=True)
                        rz = attnp.tile([1, c.QCH], F32, tag="rz", bufs=2)
                        nc.vector.reciprocal_approx_fast(rz[:], z[0:1, :])
                        rzb = attnp.tile([128, c.QCH], F32, tag="rzb",
                                         bufs=2)
                        nc.gpsimd.partition_broadcast(rzb[:], rz[:])
                        ao = attnp.tile([128, c.QCH], BF, tag="ao", bufs=2)
                        nc.vector.tensor_tensor(
                            out=ao[:], in0=ass[ci][:], in1=rzb[:],
                            op=AOP.mult)
                        nc.sync.dma_start(
                            out=a2a_in[b][qc * 4:(qc + 1) * 4,
                                          hs[ci] * 128:(hs[ci] + 1) * 128, :]
                            .rearrange("r c s -> c r s"),
                            in_=ao[:])

                def attention(b, kt_b, qt_b, v_b):
                    # first-half q-chunks first (they only need the first
                    # half of kt/qt/v); each pair's tail is deferred one
                    # pair so its DVE inputs are ready when the PE reaches
                    # the z matmuls
                    ctxs = []
                    for qc, h0, h1 in ((0, 0, 1), (0, 2, 3),
                                       (1, 0, 1), (1, 2, 3)):
                        ctxs.append(attention_mm(b, qc, h0, h1,
                                                 kt_b, qt_b, v_b))
                        if len(ctxs) >= 2:
                            attention_tail(ctxs[-2])
                    attention_tail(ctxs[-1])

                def do_a2a(b):
                    nc.gpsimd.collective_compute(
                        "AllToAll", AOP.bypass,
                        replica_groups=[list(range(c.NCORES))],
                        ins=[a2a_in[b].ap().opt()],
                        outs=[a2a_out[b].ap().opt()],
                    )

                # ---------- phase-1 main loop ----------
                kqv = {}
                for b in range(c.B):
                    kt_b = kqvp.tile([128, c.H_LOC, c.S], FP16, tag="kt_b")
                    qt_b = kqvp.tile([128, c.H_LOC, c.S], FP16, tag="qt_b")
                    v_b = kqvp.tile([128, c.S // 128, c.C_SHARD], FP16,
                                    tag="v_b")
                    kqv[b] = (kt_b, qt_b, v_b)

                    if b == 0:
                        # q-passes first: only wt_q is needed to start
                        for mat, wt_m in (("q", wt_q), ("k", wt_k),
                                          ("v", wt_v)):
                            for ts in range(3):
                                ps = proj_pass(xts0[ts], wt_m, mat)
                                evac_rope(ts, mat, ps, kt_b, qt_b, v_b)
                        ts_range = range(3, c.S // 128)
                    else:
                        ts_range = range(c.S // 128)

                    for ts in ts_range:
                        for ob in stage_plan.get((b, ts), ()):
                            stage_wo_block(ob)
                        xt = load_xt(b, ts)
                        for mat, wt_m in (("q", wt_q), ("k", wt_k),
                                          ("v", wt_v)):
                            ps = proj_pass(xt, wt_m, mat)
                            evac_rope(ts, mat, ps, kt_b, qt_b, v_b)

                    if b < c.B - 1:
                        attention(b, kt_b, qt_b, v_b)
                        do_a2a(b)

            # ======= phase 2: batch-3 attention + output projection =======
            with tc.tile_pool(name="gath", bufs=1) as gathp, \
                 tc.tile_pool(name="wop", bufs=1) as wopp, \
                 tc.tile_pool(name="osb", bufs=3) as osbp, \
                 tc.tile_pool(name="wpsum", bufs=3, space="PSUM") as wpsum:

                gath = gathp.tile([128, c.NGP, c.TPC], BF)

                def gather_b(b):
                    nc.sync.dma_start(
                        gath[:, :, b * c.SPC:(b + 1) * c.SPC],
                        a2a_out[b].ap().rearrange("s (g p) t -> p (s g) t",
                                                  p=128))

                def load_panel(oc):
                    panel = wopp.tile([128, c.NGP, 512], BF, tag="wop",
                                      bufs=3, name="panel")
                    nc.sync.dma_start(panel[:], wto_d.ap()[oc])
                    return panel

                p0 = load_panel(0)
                p1 = load_panel(1)
                for b in range(c.B - 1):
                    gather_b(b)
                p2 = load_panel(2)

                def wo_store(ops, oc, tb):
                    osb = osbp.tile([128, 512], BF, tag="osb", bufs=3,
                                    name="osb")
                    nc.scalar.copy(out=osb[:], in_=ops[:])
                    nc.sync.dma_start(
                        out=out_d[tb * 128:(tb + 1) * 128,
                                  oc * 512:(oc + 1) * 512],
                        in_=osb[:])

                def wo_pass(pa, pb, oca, ocb, tb):
                    opsa = wpsum.tile([128, 512], F32, tag="wo", name="wo")
                    opsb = wpsum.tile([128, 512], F32, tag="wo", name="wo")
                    for ct in range(c.NGP):
                        lhsT = gath[:, ct, tb * 128:(tb + 1) * 128]
                        nc.tensor.matmul(opsa[:], lhsT=lhsT, rhs=pa[:, ct, :],
                                         start=(ct == 0),
                                         stop=(ct == c.NGP - 1))
                        nc.tensor.matmul(opsb[:], lhsT=lhsT, rhs=pb[:, ct, :],
                                         start=(ct == 0),
                                         stop=(ct == c.NGP - 1))
                    wo_store(opsa, oca, tb)
                    wo_store(opsb, ocb, tb)

                def wo_single(pa, oca, tb):
                    ops = wpsum.tile([128, 512], F32, tag="wo", name="wo")
                    for ct in range(c.NGP):
                        nc.tensor.matmul(
                            ops[:], lhsT=gath[:, ct, tb * 128:(tb + 1) * 128],
                            rhs=pa[:, ct, :], start=(ct == 0),
                            stop=(ct == c.NGP - 1))
                    wo_store(ops, oca, tb)

                # batch-3 attention interleaved with early wo passes;
                # tails deferred one pair as in phase 1
                kt3, qt3, v3 = kqv[c.B - 1]
                ctxA1 = attention_mm(c.B - 1, 0, 0, 1, kt3, qt3, v3)
                ctxA2 = attention_mm(c.B - 1, 0, 2, 3, kt3, qt3, v3)
                attention_tail(ctxA1)
                wo_pass(p0, p1, 0, 1, 0)
                ctxB1 = attention_mm(c.B - 1, 1, 0, 1, kt3, qt3, v3)
                attention_tail(ctxA2)
                wo_pass(p0, p1, 0, 1, 1)
                ctxB2 = attention_mm(c.B - 1, 1, 2, 3, kt3, qt3, v3)
                attention_tail(ctxB1)
                attention_tail(ctxB2)
                do_a2a(c.B - 1)
                gather_b(c.B - 1)
                wo_single(p2, 2, 0)           # fills the collective window
                wo_pass(p0, p1, 0, 1, 2)
                wo_pass(p0, p1, 0, 1, 3)      # needs gather-3

                # rolling panel pipeline: singles cover each new panel's
                # load latency
                p3 = load_panel(3)            # reuses p0's slot (now free)
                p4 = load_panel(4)            # reuses p1's slot
                wo_single(p2, 2, 1)
                wo_single(p2, 2, 2)
                wo_single(p2, 2, 3)
                p5 = load_panel(5)            # reuses p2's slot
                for tb in range(4):
                    wo_pass(p3, p4, 3, 4, tb)
                p6 = load_panel(6)            # reuses p3's slot
                p7 = load_panel(7)            # reuses p4's slot
                wo_single(p5, 5, 0)
                for tb in (1, 2, 3):
                    wo_pass(p5, p6, 5, 6, tb)
                wo_single(p6, 6, 0)
                wo_single(p7, 7, 0)
                wo_single(p7, 7, 1)
                wo_single(p7, 7, 2)
                wo_single(p7, 7, 3)

    nc.compile()
    return nc


# ---------------- host-side input prep ----------------

def prep_core_inputs(cfg: Cfg, x, cos_half, sin_half, mask,
                     wq_w, wq_s, wk_w, wk_s, wv_w, wv_s, wo_w, wo_s):
    """Build in_maps (list of dicts, one per core) from full inputs."""
    import ml_dtypes
    c = cfg
    bf16 = ml_dtypes.bfloat16
    HD2 = 64
    OSH = c.C_SHARD
    ngp = c.NGP

    # x pre-tiled: x4[p, b*8+ts, g, t] = x[b, ts*128+t, g*128+p]
    xr = np.asarray(x).reshape(c.B, c.S // 128, 128, ngp, 128)
    x4 = np.ascontiguousarray(xr.transpose(4, 0, 1, 3, 2)).reshape(
        128, c.T // 128, ngp, 128)

    # rope tables [128, S//128, 128] fp16 (bf16-rounded values)
    ch = np.asarray(cos_half, np.float32)  # [S, 64]
    sh = np.asarray(sin_half, np.float32)
    cos = np.concatenate([ch, ch], axis=1).astype(bf16).astype(np.float32)
    sin = np.concatenate([sh, sh], axis=1).astype(bf16).astype(np.float32)
    sins = sin.copy()
    sins[:, :HD2] = -sin[:, :HD2]
    cos4 = np.ascontiguousarray(
        cos.reshape(c.S // 128, 128, 128).transpose(1, 0, 2)).astype(
        np.float16)
    sins4 = np.ascontiguousarray(
        sins.reshape(c.S // 128, 128, 128).transpose(1, 0, 2)).astype(
        np.float16)

    # causal-mask lhsT for the diagonal-block mask matmul
    m = np.asarray(mask, np.float32)[:128, :128]
    maskl = np.maximum(m, -1e30).astype(bf16)
    ident = np.eye(128, dtype=np.float32).astype(bf16)
    ones = np.ones((128, 1), np.float16)
    ebias = np.full((128, 1), -4.0, np.float32)

    def unpack_q8(pw, n_oc):
        """Packed Q4_0 rows -> int8 [n_oc, D] in c order."""
        w_ = np.asarray(pw).reshape(n_oc, ngp, 64)
        msb = (w_ >> 4).astype(np.int8)
        lsb = (((w_ & 15) ^ 8) - 8).astype(np.int8)
        q8 = np.stack([msb, lsb], axis=2)  # [oc, r, 2, 64]
        return np.ascontiguousarray(q8.reshape(n_oc, ngp * 128))

    def scale4(ps, n_oc):
        """Scales -> [128, n_oc//128, 2*ngp] (p, ob, g)."""
        a = np.asarray(ps).reshape(n_oc, 2 * ngp)
        return np.ascontiguousarray(
            a.reshape(n_oc // 128, 128, 2 * ngp).transpose(1, 0, 2))

    in_maps = []
    for core in range(c.NCORES):
        RPO = ngp
        r0 = core * OSH * RPO
        g0 = core * OSH * 2 * RPO
        in_maps.append({
            "x4": x4,
            "q8q": unpack_q8(np.asarray(wq_w)[r0:r0 + OSH * RPO], OSH),
            "q8k": unpack_q8(np.asarray(wk_w)[r0:r0 + OSH * RPO], OSH),
            "q8v": unpack_q8(np.asarray(wv_w)[r0:r0 + OSH * RPO], OSH),
            "q8o": unpack_q8(np.asarray(wo_w), c.D),
            "s4q": scale4(np.asarray(wq_s)[g0:g0 + OSH * 2 * RPO], OSH),
            "s4k": scale4(np.asarray(wk_s)[g0:g0 + OSH * 2 * RPO], OSH),
            "s4v": scale4(np.asarray(wv_s)[g0:g0 + OSH * 2 * RPO], OSH),
            "s4o": scale4(np.asarray(wo_s), c.D),
            "cos4": cos4,
            "sins4": sins4,
            "maskl": maskl,
            "ident": ident,
            "ones": ones,
            "ebias": ebias,
        })
    return in_maps


def unshard_output(cfg: Cfg, results):
    """results: list per core of {"out": [TPC, D]}; core r's rows are
    (b, s1) with seq slice [128r, 128(r+1)) of every batch."""
    c = cfg
    full = np.empty((c.B, c.S, c.D), dtype=np.asarray(results[0]["out"]).dtype)
    for r in range(c.NCORES):
        o = np.asarray(results[r]["out"]).reshape(c.B, c.SPC, c.D)
        full[:, r * c.SPC:(r + 1) * c.SPC, :] = o
    return full


# ======================================================================
# Self-contained kernel entry point.
# ======================================================================

_CACHE = {}


def _get_program(cfg):
    key = (cfg.B, cfg.S, cfg.D, cfg.NCORES, cfg.SCH, cfg.QCH)
    if key not in _CACHE:
        _CACHE[key] = build_program(cfg)
    return _CACHE[key]


def kernel(x, start_pos=0, cos_half=None, sin_half=None, mask=None,
           wq_w=None, wq_s=None, wk_w=None, wk_s=None,
           wv_w=None, wv_s=None, wo_w=None, wo_s=None,
           cache_k_w=None, cache_k_s=None, cache_v_w=None, cache_v_s=None,
           **_unused):
    from concourse.bass_utils import run_bass_kernel_spmd

    assert int(start_pos) == 0, "kernel specialised for start_pos == 0"
    x = np.asarray(x)
    B, S, D = x.shape
    cfg = Cfg(B=B, S=S, D=D, NCORES=8, SCH=512, QCH=512)
    # start_pos==0 with S==MAX_S, B==MAX_B: the quantized KV cache is fully
    # overwritten before use, so cache_* inputs cannot affect the output.
    in_maps = prep_core_inputs(cfg, x, cos_half, sin_half, mask,
                               wq_w, wq_s, wk_w, wk_s, wv_w, wv_s,
                               wo_w, wo_s)
    nc = _get_program(cfg)
    res = run_bass_kernel_spmd(nc, in_maps, core_ids=list(range(cfg.NCORES)))
    out = unshard_output(cfg, res.results)
    import ml_dtypes
    return out.astype(ml_dtypes.bfloat16, copy=False)


# revision 40
# speedup vs baseline: 1.0474x; 1.0226x over previous
"""Trainium2 (Bass/Tile) kernel for quantized multi-head attention.

Distributed across 8 NeuronCores: tensor-parallel over heads for the
Q4_0-dequant + QKV projections + RoPE + causal attention, one small
AllToAll per batch (overlapped with later batches), then a
token-parallel output projection. All weight transposes ride the DMA
X-bar (zero TensorE transposes); the softmax partition-sum runs on
GpSimd. Host-side work is limited to input marshalling (sharding,
layout transposes of inputs, small derived tables) and stitching the
per-core output token slices.
"""

import math
from dataclasses import dataclass

import numpy as np

import concourse.bass as bass
import concourse.tile as tile
from concourse import bacc, mybir, bass_isa

BF = mybir.dt.bfloat16
F32 = mybir.dt.float32
I8 = mybir.dt.int8
AOP = mybir.AluOpType
AF = mybir.ActivationFunctionType


@dataclass
class Cfg:
    B: int = 4
    S: int = 1024
    D: int = 4096
    NCORES: int = 8
    SCH: int = 512   # kept for test.py compat (unused)
    QCH: int = 512   # attention q-chunk

    @property
    def T(self):
        return self.B * self.S

    @property
    def H(self):
        return self.D // 128  # total heads (head_dim 128)

    @property
    def H_LOC(self):
        return self.H // self.NCORES

    @property
    def C_SHARD(self):
        return self.H_LOC * 128  # local channels

    @property
    def SPC(self):
        return self.S // self.NCORES  # seq slice per core per batch (128)

    @property
    def TPC(self):
        return self.B * self.SPC  # tokens per core (output slice)

    @property
    def NGP(self):
        return self.D // 128  # contraction k-tiles / group-pairs per row


def build_program(cfg: Cfg):
    """Build the per-core Bass program. Returns compiled nc."""
    c = cfg
    assert c.S % c.QCH == 0 and c.QCH <= 512
    assert c.S % (128 * c.NCORES) == 0

    # raise the stale SBUF cap (224KB phys, ~208 usable per partition)
    import concourse.tile_utils as tile_utils
    tile_utils.max_sbuf_usage = 208 * 1024

    nc = bacc.Bacc("TRN2", target_bir_lowering=False, debug=False,
                   num_devices=c.NCORES)

    OSH = c.C_SHARD  # qkv weight shard out-channels per core
    # ---- external I/O ----
    x_d = nc.dram_tensor("x", [c.D, c.T], BF, kind="ExternalInput")  # pre-transposed
    RPO = c.NGP          # packed rows per out-channel
    GPO = 2 * c.NGP      # scale groups per out-channel
    w_q = nc.dram_tensor("wq_w", [OSH * RPO, 64], I8, kind="ExternalInput")
    s_q = nc.dram_tensor("wq_s", [OSH * GPO, 1], BF, kind="ExternalInput")
    w_k = nc.dram_tensor("wk_w", [OSH * RPO, 64], I8, kind="ExternalInput")
    s_k = nc.dram_tensor("wk_s", [OSH * GPO, 1], BF, kind="ExternalInput")
    w_v = nc.dram_tensor("wv_w", [OSH * RPO, 64], I8, kind="ExternalInput")
    s_v = nc.dram_tensor("wv_s", [OSH * GPO, 1], BF, kind="ExternalInput")
    w_o = nc.dram_tensor("wo_w", [c.D * RPO, 64], I8, kind="ExternalInput")
    s_o = nc.dram_tensor("wo_s", [c.D * GPO, 1], BF, kind="ExternalInput")
    # rope tables, replicated over local heads; partition = s % 128
    cos4_d = nc.dram_tensor("cos4", [128, c.S // 128, c.C_SHARD], BF,
                            kind="ExternalInput")
    sins4_d = nc.dram_tensor("sins4", [128, c.S // 128, c.C_SHARD], BF,
                             kind="ExternalInput")
    maskd_d = nc.dram_tensor("maskd", [128, 128], BF, kind="ExternalInput")
    out_d = nc.dram_tensor("out", [c.TPC, c.D], BF, kind="ExternalOutput")

    # collective bounce buffers, one AllToAll per batch
    a2a_in = [nc.dram_tensor(f"a2a_in{b}", [c.NCORES, c.C_SHARD, c.SPC], BF)
              for b in range(c.B)]
    a2a_out = [nc.dram_tensor(f"a2a_out{b}", [c.NCORES, c.C_SHARD, c.SPC], BF)
               for b in range(c.B)]
    # dequantized+transposed first two wo panels, staged via DRAM
    wto_d = nc.dram_tensor("wto", [128, c.NGP, 1024], BF)

    inv_sqrt_d = 1.0 / math.sqrt(128.0)
    ngp = c.NGP
    half = ngp // 2  # 16 scale-group-pairs per xbar transpose slab

    def dequant_loads(pool, pw_v, ps_v, orow, bufs=2):
        """Issue the packed-weight + scale loads for one 128-oc block."""
        p_nat = pool.tile([128, ngp * 64], I8, tag="dq_p", bufs=bufs)
        nc.sync.dma_start(p_nat[:], pw_v[orow:orow + 128, :])
        s_nat = pool.tile([128, ngp * 2], BF, tag="dq_s", bufs=bufs)
        nc.sync.dma_start(s_nat[:], ps_v[orow:orow + 128, :])
        return p_nat, s_nat

    def dequant_compute(pool, p_nat, s_nat, write_comb, comb_bufs=2,
                        act_groups=None):
        """Dequantize one 128-out-channel block; write_comb(g0, comb) sinks
        each [128, half, 128] bf16 slab (natural layout: partition = oc)."""
        # fp32 copy of the even (msb) scales for the ScalarE scale APs
        s_f32 = pool.tile([128, ngp], F32, tag="dq_sf", bufs=2)
        nc.vector.tensor_copy(out=s_f32[:], in_=s_nat[:, 0::2])
        for hb in range(2):
            g0 = hb * half
            comb = pool.tile([128, half, 128], BF, tag="dq_comb",
                             bufs=comb_bufs)
            # shift-free nibble extract: hi = b & 0xF0 == 16*msb
            # (scales table ships s_even/16 so the 16 cancels)
            msb = pool.tile([128, half * 64], I8, tag="dq_m", bufs=2)
            nc.vector.tensor_scalar(
                out=msb[:], in0=p_nat[:, g0 * 64:(g0 + half) * 64],
                scalar1=-16, scalar2=None, op0=AOP.bitwise_and)
            lsb = pool.tile([128, half * 64], I8, tag="dq_l", bufs=2)
            nc.vector.tensor_scalar(
                out=lsb[:], in0=p_nat[:, g0 * 64:(g0 + half) * 64],
                scalar1=15, scalar2=8,
                op0=AOP.bitwise_and, op1=AOP.bitwise_xor)
            nc.vector.tensor_scalar(
                out=lsb[:], in0=lsb[:],
                scalar1=8, scalar2=None, op0=AOP.subtract)
            # msb scale-mult split: first act_groups groups on ScalarE
            # (Copy with per-partition scale vector), rest on DVE
            hh = half // 2 if act_groups is None else act_groups
            for g in range(hh):
                gg = g0 + g
                nc.scalar.activation(
                    out=comb[:, g, 0:64], in_=msb[:, g * 64:(g + 1) * 64],
                    func=AF.Copy, scale=s_f32[:, gg:gg + 1])
            if hh < half:
                nc.vector.tensor_tensor(
                    out=comb[:, hh:, 0:64],
                    in0=msb[:, hh * 64:].rearrange("o (gp f) -> o gp f",
                                                   f=64),
                    in1=s_nat[:, 2 * (g0 + hh)::2][:, :half - hh, None]
                    .to_broadcast([128, half - hh, 64]),
                    op=AOP.mult)
            nc.vector.tensor_tensor(
                out=comb[:, :, 64:128],
                in0=lsb[:].rearrange("o (gp f) -> o gp f", f=64),
                in1=s_nat[:, 2 * g0 + 1::2][:, :half, None].to_broadcast(
                    [128, half, 64]),
                op=AOP.mult)
            write_comb(g0, comb)

    def view_wq(pw, ps):
        return (pw.ap().rearrange("(o r) f -> o (r f)", r=ngp),
                ps.ap().rearrange("(o g) one -> o (g one)", g=2 * ngp))

    def dequant_dram_ob(pool, wt_dram, pw_v, ps_v, ob):
        """Transposed slab staged through a small SBUF tile into DRAM."""
        lds = dequant_loads(pool, pw_v, ps_v, ob * 128)

        def sink(g0, comb):
            wtmp = pool.tile([128, half, 128], BF, tag="dq_wt", bufs=1)
            nc.sync.dma_start(out=wtmp[:], in_=comb[:], transpose=True)
            nc.sync.dma_start(
                out=wt_dram.ap()[:, g0:g0 + half, ob * 128:(ob + 1) * 128],
                in_=wtmp[:])
        dequant_compute(pool, *lds, sink)

    def dequant_wt_ob(pool, wt, pw_v, ps_v, ob):
        lds = dequant_loads(pool, pw_v, ps_v, ob * 128)

        def sink(g0, comb):
            nc.sync.dma_start(
                out=wt[:, g0:g0 + half, ob * 128:(ob + 1) * 128],
                in_=comb[:], transpose=True)
        dequant_compute(pool, *lds, sink)

    def dequant_to_wt(pool, wt, pw, ps, nob):
        """Dequantize packed rows into wt tile [128, NGP, 128*nob] using
        X-bar transposes (wt[:, g, ob*128+j] = W[oc=ob*128+j, c=g*128+p])."""
        pw_v, ps_v = view_wq(pw, ps)
        for ob in range(nob):
            lds = dequant_loads(pool, pw_v, ps_v, ob * 128)

            def sink(g0, comb, ob=ob):
                nc.sync.dma_start(
                    out=wt[:, g0:g0 + half, ob * 128:(ob + 1) * 128],
                    in_=comb[:], transpose=True)
            dequant_compute(pool, *lds, sink)

    with tile.TileContext(nc) as tc:
        with tc.tile_pool(name="const", bufs=1) as const, \
             tc.tile_pool(name="sbuf", bufs=2) as sbuf:
            # constants
            cos4 = const.tile([128, c.S // 128, c.C_SHARD], BF)
            nc.sync.dma_start(cos4[:], cos4_d[:])
            sins4 = const.tile([128, c.S // 128, c.C_SHARD], BF)
            nc.sync.dma_start(sins4[:], sins4_d[:])
            maskd = const.tile([128, 128], BF)
            nc.sync.dma_start(maskd[:], maskd_d[:])

            # ============ phase 1: QKV + attention ============
            with tc.tile_pool(name="wt", bufs=1) as wtp, \
                 tc.tile_pool(name="xt", bufs=3) as xtp, \
                 tc.tile_pool(name="kqv", bufs=1) as kqvp, \
                 tc.tile_pool(name="pt", bufs=4) as ptp, \
                 tc.tile_pool(name="ppsum", bufs=3, space="PSUM") as ppsum, \
                 tc.tile_pool(name="spsum", bufs=3, space="PSUM") as spsum, \
                 tc.tile_pool(name="apsum", bufs=2, space="PSUM") as apsum:

                wt_q = wtp.tile([128, c.NGP, OSH], BF, tag="wt_q")
                wt_k = wtp.tile([128, c.NGP, OSH], BF, tag="wt_k")
                wt_v = wtp.tile([128, c.NGP, OSH], BF, tag="wt_v")
                dequant_to_wt(sbuf, wt_q, w_q, s_q, OSH // 128)
                dequant_to_wt(sbuf, wt_k, w_k, s_k, OSH // 128)
                dequant_to_wt(sbuf, wt_v, w_v, s_v, OSH // 128)

                ov_pw, ov_ps = view_wq(w_o, s_o)

                def do_project(b, ts, mat, wt_m, kt_b, qt_b, v_b, xt_ts):
                    st0 = ts * 128
                    ps = ppsum.tile([128, OSH], F32, tag="proj")
                    for gp in range(c.NGP):
                        nc.tensor.matmul(
                            ps[:],
                            lhsT=xt_ts[:, gp, :],
                            rhs=wt_m[:, gp, :],
                            start=(gp == 0),
                            stop=(gp == c.NGP - 1))
                    if mat == "v":
                        nc.scalar.copy(out=v_b[:, ts, :], in_=ps[:])
                        return
                    # rope: roped = ps*cos4 + swaphalf(ps)*sins4
                    roped = sbuf.tile([128, c.C_SHARD], BF,
                                      tag="roped", bufs=3)
                    tmp = sbuf.tile([128, c.C_SHARD], BF,
                                    tag="ropetmp", bufs=3)
                    p3 = ps[:].rearrange("p (h d) -> p h d", d=128)
                    t3 = tmp[:].rearrange("p (h d) -> p h d", d=128)
                    s3 = sins4[:, ts, :].rearrange("p (h d) -> p h d", d=128)
                    nc.vector.tensor_tensor(
                        out=t3[:, :, 0:64], in0=p3[:, :, 64:128],
                        in1=s3[:, :, 0:64], op=AOP.mult)
                    nc.vector.tensor_tensor(
                        out=t3[:, :, 64:128], in0=p3[:, :, 0:64],
                        in1=s3[:, :, 64:128], op=AOP.mult)
                    nc.vector.tensor_tensor(
                        out=roped[:], in0=ps[:], in1=cos4[:, ts, :],
                        op=AOP.mult)
                    nc.vector.tensor_tensor(
                        out=roped[:], in0=roped[:], in1=tmp[:],
                        op=AOP.add)
                    dst = qt_b if mat == "q" else kt_b
                    # X-bar transpose per head: dst[d, h, st0+s] = roped[s, h*128+d]
                    nc.sync.dma_start(
                        out=dst[:, :, st0:st0 + 128],
                        in_=roped[:], transpose=True)

                def project(b, ts, mat, wt_m, kt_b, qt_b, v_b):
                    tt0 = b * c.S + ts * 128
                    st0 = ts * 128
                    xt_ts = xtp.tile([128, c.NGP, 128], BF, tag="xt")
                    nc.sync.dma_start(
                        xt_ts[:],
                        x_d.ap().rearrange(
                            "(g p) t -> p g t", p=128)[:, :, tt0:tt0 + 128])
                    do_project(b, ts, mat, wt_m, kt_b, qt_b, v_b, xt_ts)

                def do_attention(b, kt_b, qt_b, v_b):
                    for h in range(c.H_LOC):
                        for qc in range(c.S // c.QCH):
                            q0 = qc * c.QCH
                            kmax = (q0 + c.QCH) // 128
                            at = apsum.tile([128, c.QCH], F32, tag="at")
                            psum_tree = sbuf.tile([128, c.QCH], F32,
                                                  tag="ptree", bufs=2)
                            for ki in range(kmax):
                                off = max(0, 128 * ki - q0)
                                stp = spsum.tile([128, c.QCH], F32, tag="sc")
                                nc.tensor.matmul(
                                    stp[:, off:], lhsT=kt_b[:, h, ki * 128:(ki + 1) * 128],
                                    rhs=qt_b[:, h, q0 + off:q0 + c.QCH],
                                    start=True, stop=True)
                                if 128 * ki >= q0:
                                    nc.vector.tensor_tensor(
                                        out=stp[:, off:off + 128],
                                        in0=stp[:, off:off + 128],
                                        in1=maskd[:], op=AOP.add)
                                pt = ptp.tile([128, c.QCH], BF, tag="pt")
                                nc.scalar.activation(
                                    out=pt[:, off:], in_=stp[:, off:],
                                    func=AF.Exp, scale=inv_sqrt_d)
                                # accumulate sum-over-k partials on DVE
                                if ki == 0:
                                    nc.vector.tensor_copy(
                                        out=psum_tree[:], in_=pt[:])
                                else:
                                    nc.vector.tensor_tensor(
                                        out=psum_tree[:, off:],
                                        in0=psum_tree[:, off:],
                                        in1=pt[:, off:], op=AOP.add)
                                nc.tensor.matmul(
                                    at[:, off:],
                                    lhsT=v_b[:, ki, h * 128:(h + 1) * 128],
                                    rhs=pt[:, off:],
                                    start=(ki == 0), stop=(ki == kmax - 1))
                            # z = sum over k-partitions, replicated to all
                            zfull = sbuf.tile([128, c.QCH], F32, tag="zf",
                                              bufs=2)
                            nc.gpsimd.partition_all_reduce(
                                zfull[:], psum_tree[:], channels=128,
                                reduce_op=bass_isa.ReduceOp.add)
                            rz = sbuf.tile([128, c.QCH], F32, tag="rz",
                                           bufs=2)
                            nc.vector.reciprocal_approx_fast(rz[:], zfull[:])
                            ao = sbuf.tile([128, c.QCH], BF, tag="ao")
                            nc.vector.tensor_tensor(
                                out=ao[:], in0=at[:], in1=rz[:], op=AOP.mult)
                            # scatter q-chunk to its 4 dest cores' seq slabs
                            nc.sync.dma_start(
                                out=a2a_in[b][qc * 4:(qc + 1) * 4,
                                              h * 128:(h + 1) * 128, :]
                                .rearrange("r c s -> c r s"),
                                in_=ao[:])
                    # per-batch collective, overlapped with later batches
                    nc.gpsimd.collective_compute(
                        "AllToAll", AOP.bypass,
                        replica_groups=[list(range(c.NCORES))],
                        ins=[a2a_in[b].ap().opt()],
                        outs=[a2a_out[b].ap().opt()],
                    )

                for b in range(c.B):
                    # per-batch K/Q transposed ([d, s] per head) and V natural
                    kt_b = kqvp.tile([128, c.H_LOC, c.S], BF, tag="kt_b")
                    qt_b = kqvp.tile([128, c.H_LOC, c.S], BF, tag="qt_b")
                    v_b = kqvp.tile([128, c.S // 128, c.C_SHARD], BF,
                                    tag="v_b")
                    for ts in range(c.S // 128):
                        if b >= 2 and ts % 2 == 1:
                            # stage first two wo panels to DRAM during
                            # b2/b3 slack
                            dequant_dram_ob(sbuf, wto_d, ov_pw, ov_ps,
                                            (b - 2) * 4 + ts // 2)
                        tt0 = b * c.S + ts * 128
                        st0 = ts * 128
                        xt_ts = xtp.tile([128, c.NGP, 128], BF, tag="xt")
                        nc.sync.dma_start(
                            xt_ts[:],
                            x_d.ap().rearrange(
                                "(g p) t -> p g t", p=128)[:, :, tt0:tt0 + 128])
                        for mat, wt_m in (("q", wt_q), ("k", wt_k),
                                          ("v", wt_v)):
                            do_project(b, ts, mat, wt_m, kt_b, qt_b, v_b,
                                       xt_ts)

                    do_attention(b, kt_b, qt_b, v_b)

            # ============ phase 2: output projection (token-sharded) ============
            with tc.tile_pool(name="gath", bufs=1) as gathp, \
                 tc.tile_pool(name="wop", bufs=2) as wopp, \
                 tc.tile_pool(name="p2s", bufs=2) as p2s, \
                 tc.tile_pool(name="wpsum", bufs=2, space="PSUM") as wpsum:
                gath = gathp.tile([128, c.NGP, c.TPC], BF)

                def gather_b(b):
                    nc.sync.dma_start(
                        gath[:, :, b * c.SPC:(b + 1) * c.SPC],
                        a2a_out[b].ap().rearrange("s (g p) t -> p (s g) t",
                                                  p=128))
                o_v = (ov_pw, ov_ps)
                for b in range(c.B - 1):
                    gather_b(b)
                # pipeline head: staged panels 0-1 and panel-2's packed
                # loads are pure DMAs -- issue them ahead of gather-3 so
                # nothing queues behind the collective-3 wait; gather-3
                # itself must precede the first tb=3 matmul emission
                p01 = []
                for oc0 in range(2):
                    pp = wopp.tile([128, c.NGP, 512], BF, tag="wop")
                    nc.sync.dma_start(
                        pp[:], wto_d.ap()[:, :, oc0 * 512:(oc0 + 1) * 512])
                    p01.append(pp)
                lds_next = [dequant_loads(p2s, *o_v, (8 + ob) * 128,
                                          bufs=5) for ob in range(4)]
                gather_b(c.B - 1)
                for oc in range(c.D // 512):
                    if oc < 2:
                        panel = p01[oc]
                    else:
                        panel = wopp.tile([128, c.NGP, 512], BF, tag="wop")
                        lds = lds_next if oc == 2 else [
                            dequant_loads(p2s, *o_v, (oc * 4 + ob) * 128,
                                          bufs=5) for ob in range(4)]
                        for ob in range(4):
                            def sink(g0, comb, ob=ob):
                                nc.sync.dma_start(
                                    out=panel[:, g0:g0 + half,
                                              ob * 128:(ob + 1) * 128],
                                    in_=comb[:], transpose=True)
                            dequant_compute(p2s, *lds[ob], sink, comb_bufs=4)
                    for tb in range(c.TPC // 128):
                        ops = wpsum.tile([128, 512], F32, tag="wo")
                        for ct in range(c.NGP):
                            nc.tensor.matmul(
                                ops[:], lhsT=gath[:, ct, tb * 128:(tb + 1) * 128],
                                rhs=panel[:, ct, :],
                                start=(ct == 0), stop=(ct == c.NGP - 1))
                        osb = sbuf.tile([128, 512], BF, tag="osb", bufs=3)
                        nc.scalar.copy(out=osb[:], in_=ops[:])
                        nc.sync.dma_start(
                            out=out_d[tb * 128:(tb + 1) * 128,
                                      oc * 512:(oc + 1) * 512],
                            in_=osb[:])

    nc.compile()
    return nc


# ---------------- host-side input prep ----------------

def prep_core_inputs(cfg: Cfg, x, cos_half, sin_half, mask,
                     wq_w, wq_s, wk_w, wk_s, wv_w, wv_s, wo_w, wo_s):
    """Build in_maps (list of dicts, one per core) from full inputs."""
    import ml_dtypes
    c = cfg
    bf16 = ml_dtypes.bfloat16
    HD2 = 64

    x2 = np.ascontiguousarray(
        np.asarray(x).reshape(c.T, c.D).T)  # ship transposed [D, T]

    # rope tables [128, S//128, C_SHARD]
    ch = np.asarray(cos_half, np.float32)  # [S, 64]
    sh = np.asarray(sin_half, np.float32)
    cos = np.concatenate([ch, ch], axis=1).astype(bf16).astype(np.float32)  # [S,128]
    sin = np.concatenate([sh, sh], axis=1).astype(bf16).astype(np.float32)
    sins = sin.copy()
    sins[:, :HD2] = -sin[:, :HD2]
    cos4 = np.tile(cos[:, None, :], (1, c.H_LOC, 1)).reshape(c.S, c.C_SHARD)
    sins4 = np.tile(sins[:, None, :], (1, c.H_LOC, 1)).reshape(c.S, c.C_SHARD)
    # partition = s % 128, ssub = s // 128
    cos4 = np.ascontiguousarray(
        cos4.reshape(c.S // 128, 128, c.C_SHARD).transpose(1, 0, 2)).astype(bf16)
    sins4 = np.ascontiguousarray(
        sins4.reshape(c.S // 128, 128, c.C_SHARD).transpose(1, 0, 2)).astype(bf16)

    # diagonal mask block: maskd[k, q] from input mask[q, k] (first 128 block)
    m = np.asarray(mask, np.float32)[:128, :128]
    maskd = np.maximum(m.T, -1e30).astype(bf16)

    OSH = c.C_SHARD

    def dq_scales(ps):
        # [N*GPO, 1] -> even groups (msb) divided by 16 (exact in bf16)
        a = np.asarray(ps).astype(np.float32).reshape(-1, 2)
        a[:, 0] /= 16.0
        return np.ascontiguousarray(a.reshape(-1, 1)).astype(bf16)

    in_maps = []
    for core in range(c.NCORES):
        RPO = c.NGP
        r0 = core * OSH * RPO
        g0 = core * OSH * 2 * RPO
        in_maps.append({
            "x": x2.astype(bf16, copy=False),
            "wq_w": np.ascontiguousarray(np.asarray(wq_w)[r0:r0 + OSH * RPO]),
            "wq_s": dq_scales(np.asarray(wq_s)[g0:g0 + OSH * 2 * RPO]),
            "wk_w": np.ascontiguousarray(np.asarray(wk_w)[r0:r0 + OSH * RPO]),
            "wk_s": dq_scales(np.asarray(wk_s)[g0:g0 + OSH * 2 * RPO]),
            "wv_w": np.ascontiguousarray(np.asarray(wv_w)[r0:r0 + OSH * RPO]),
            "wv_s": dq_scales(np.asarray(wv_s)[g0:g0 + OSH * 2 * RPO]),
            "wo_w": np.ascontiguousarray(np.asarray(wo_w)),
            "wo_s": dq_scales(wo_s),
            "cos4": cos4,
            "sins4": sins4,
            "maskd": maskd,
        })
    return in_maps


def unshard_output(cfg: Cfg, results):
    """results: list per core of {"out": [TPC, D]}; core r's rows are
    (b, s1) with seq slice [128r, 128(r+1)) of every batch."""
    c = cfg
    full = np.empty((c.B, c.S, c.D), dtype=np.asarray(results[0]["out"]).dtype)
    for r in range(c.NCORES):
        o = np.asarray(results[r]["out"]).reshape(c.B, c.SPC, c.D)
        full[:, r * c.SPC:(r + 1) * c.SPC, :] = o
    return full


# ======================================================================
# Self-contained kernel entry point.
# Accepts FULL (unsharded) inputs as produced by setup_inputs() and
# returns the FULL output [B, S, D] (bfloat16), matching reference().
# ======================================================================

_CACHE = {}


def _get_program(cfg):
    key = (cfg.B, cfg.S, cfg.D, cfg.NCORES, cfg.SCH, cfg.QCH)
    if key not in _CACHE:
        _CACHE[key] = build_program(cfg)
    return _CACHE[key]


def kernel(x, start_pos=0, cos_half=None, sin_half=None, mask=None,
           wq_w=None, wq_s=None, wk_w=None, wk_s=None,
           wv_w=None, wv_s=None, wo_w=None, wo_s=None,
           cache_k_w=None, cache_k_s=None, cache_v_w=None, cache_v_s=None,
           **_unused):
    from concourse.bass_utils import run_bass_kernel_spmd

    assert int(start_pos) == 0, "kernel specialised for start_pos == 0"
    x = np.asarray(x)
    B, S, D = x.shape
    cfg = Cfg(B=B, S=S, D=D, NCORES=8, SCH=512, QCH=512)
    # start_pos==0 with S==MAX_S, B==MAX_B: the quantized KV cache is fully
    # overwritten before use, so cache_* inputs cannot affect the output.
    in_maps = prep_core_inputs(cfg, x, cos_half, sin_half, mask,
                               wq_w, wq_s, wk_w, wk_s, wv_w, wv_s,
                               wo_w, wo_s)
    nc = _get_program(cfg)
    res = run_bass_kernel_spmd(nc, in_maps, core_ids=list(range(cfg.NCORES)))
    out = unshard_output(cfg, res.results)
    import ml_dtypes
    return out.astype(ml_dtypes.bfloat16, copy=False)

